# revision 82
# baseline (speedup 1.0000x reference)
"""ConvFormer Trainium2 kernel v3 — fused single-pass; bf16 convs, fp8 MLP.

Data-parallel over B across 8 NeuronCores (batch element b -> core b).

Per core:
    x1 = x.T (channel-major); 3 chained masked convs (K=3,5,7) where each
    tap's contribution is masked by (chain[l+d]==chain[l]); h = LN(x + x1.T);
    out = LN(h + gelu(h@w1+b1)@w2 + b2).

v3 design (vs v2 all-bf16 at ~802 us; v3 measures ~584 us):
  - MLP in fp8 e4m3 DoubleRow (2 contraction chunks/instr, 2x PE rate);
    weights host-scaled x64 into e4m3's normal range, 1/64 folded into the
    gelu scale (mlp1) and LN2 residual drain (mlp2).  Convs must stay bf16:
    even one conv stage in fp8 measures ~3e-2 > the 2e-2 budget (conv path
    has ~unit weight in the output; the MLP path is ~0.2).
  - conv stages SBUF-resident; stage 3 L-major straight into the LN layout;
    MLP fused per 256-token block, 2-deep software pipeline (conv3 runs two
    blocks ahead of LN/MLP; PSUM: st3 ring 4 + psa/pt 2 + psB 2 = 8 banks).
  - LN1 stats+rsqrt computed one block AHEAD at scheduler high priority:
    the serial quake+Newton chain gets stretched ~3x by interleaved z-mults
    and otherwise starves the PE's h-transpose.  One Newton step (~0.17%
    rsqrt error, ~2e-3 on the metric).
  - DMA discipline: dma_start blocks its issuing queue on the hw-DMA-ring
    semaphore, so the Act queue carries no bulk DMAs (they'd head-of-line
    block the conv-stage activations); stage-1 weights land tap-by-tap
    before anything else; stage-2/3 conv + MLP weights stream in lazily
    during earlier stages on the sync/gpsimd queues.
  - z-mults multiply unshifted source windows by the mask-at-source (tap
    shift folded into the matmul read AP), keeping DVE reads aligned.
"""

import numpy as np

B, L, D = 8, 4096, 512
KS = (3, 5, 7)
EPS = 1e-5
NCORES = 8
PAD = 4            # zero-pad cols each side of channel-major tensors
PL = PAD + L + PAD
NBLK = L // 512    # 8 blocks of 512
CD = D // 128      # 4 channel chunks
LCH = L // 128     # 32 L-chunks of 128
H = 4 * D          # mlp hidden
JD = H // 128      # 16 hidden chunks
NT = sum(KS)       # 15 taps total
GELU_FUNC_NAME = "Gelu_apprx_tanh"  # jax.nn.gelu default is approximate=True

_CACHE = {}


def _build_nc(ln1_affine, ln2_affine, b2_nonzero, b1_zero):
    import concourse.bass as bass
    import concourse.tile as tile
    from concourse import bacc, mybir
    from concourse.masks import make_identity

    f32 = mybir.dt.float32
    bf16 = mybir.dt.bfloat16
    f8 = mybir.dt.float8e4
    DR = mybir.MatmulPerfMode.DoubleRow
    GELU = getattr(mybir.ActivationFunctionType, GELU_FUNC_NAME)
    IDENT = mybir.ActivationFunctionType.Identity
    ADD = mybir.AluOpType.add
    SUB = mybir.AluOpType.subtract
    MULT = mybir.AluOpType.mult
    POW = mybir.AluOpType.pow

    # global tap -> (conv index, shift d); convs use taps [0:3], [3:8], [8:15]
    conv_taps = []
    t0 = 0
    for K in KS:
        p = (K - 1) // 2
        conv_taps.append([(t0 + i, i - p) for i in range(K)])
        t0 += K
    # z-mults multiply UNSHIFTED source windows by the mask-at-source
    # (so the DVE reads stay 2-element aligned and hit the 16-bit 2X
    # perf mode); the tap shift d moves into the matmul's read AP.
    # mask-at-source for tap d is the host mask row for shift -d.
    # host mask rows are ordered by shift (-3,-2,-1,1,2,3).
    d2m = {-3: 5, -2: 4, -1: 3, 1: 2, 2: 1, 3: 0}

    nc = bacc.Bacc(None, target_bir_lowering=False)

    xcb = nc.declare_dram_parameter("xcb", [CD, 128, PL], bf16, isOutput=False)
    xb = nc.declare_dram_parameter("xb", [LCH, 128, D], f32, isOutput=False)
    masks = nc.declare_dram_parameter("masks", [6, PAD + L + PAD], bf16, isOutput=False)
    wc = nc.declare_dram_parameter("wc", [128, NT * CD * D], bf16, isOutput=False)
    cb1 = nc.declare_dram_parameter("cb1", [CD, 128], f32, isOutput=False)
    cb2 = nc.declare_dram_parameter("cb2", [CD, 128], f32, isOutput=False)
    w1 = nc.declare_dram_parameter("w1", [128, CD, H], f8, isOutput=False)
    b1c = nc.declare_dram_parameter("b1c", [JD, 128], f32, isOutput=False)
    w2 = nc.declare_dram_parameter("w2", [128, JD // 2, 2, D], f8, isOutput=False)
    if b2_nonzero:
        b2r = nc.declare_dram_parameter("b2r", [1, D], f32, isOutput=False)
    if ln1_affine:
        g1r = nc.declare_dram_parameter("g1r", [1, D], f32, isOutput=False)
        b1r = nc.declare_dram_parameter("b1r", [1, D], f32, isOutput=False)
    if ln2_affine:
        g2r = nc.declare_dram_parameter("g2r", [1, D], f32, isOutput=False)
        b2lr = nc.declare_dram_parameter("b2lr", [1, D], f32, isOutput=False)
    out = nc.declare_dram_parameter("out", [L, D], f32, isOutput=True)

    def bcast_row_ap(param, row, col0, n):
        """DMA access pattern: one DRAM row slice broadcast to 128 partitions."""
        src = param[row, col0:col0 + n]
        return bass.AP(tensor=src.tensor, offset=src.offset, ap=[[0, 128]] + list(src.ap))

    with tile.TileContext(nc) as tc:
        with (
            tc.tile_pool(name="const", bufs=1) as const,
            tc.tile_pool(name="wcp", bufs=1) as wcp,
            tc.tile_pool(name="wm", bufs=1) as wm,
            tc.tile_pool(name="s2p", bufs=1) as s2pool,
        ):
            wct = wcp.tile([128, NT * CD * D], bf16)
            w1t = wm.tile([128, CD, H], f8)
            w2t = wm.tile([128, JD // 2, 2, D], f8)

            # stage-1 weights FIRST, tap by tap (the PE consumes them in
            # tap order, ~3.4us apart); tap 0 is split scalar/sync so the
            # first matmul's weights land fastest.  After the consts the
            # scalar queue carries NO DMAs: a queued dma_start blocks on
            # the queue's hw-DMA-ring semaphore and would head-of-line
            # block the conv-stage activations sharing the Act engine.
            # Deferred weight loads go on sync+gpsimd instead.
            for t in range(3):
                for c in range(CD):
                    a0 = (t * CD + c) * D
                    nc.scalar.dma_start(out=wct[:, a0:a0 + D], in_=wc[:, a0:a0 + D])

            # deferred-load helper: large tensors are emitted in slices,
            # a few per (stage, block) iteration, alternating queues.
            def col_slices(dst, src, c0, c1, nparts):
                n = c1 - c0
                step = -(-n // nparts)
                return [
                    (lambda eng, a=c0 + s, b=min(c0 + s + step, c1):
                     eng.dma_start(out=dst[:, a:b], in_=src[:, a:b]))
                    for s in range(0, n, step)
                ]

            ident32 = const.tile([128, 128], f32)
            identb = const.tile([128, 128], bf16)
            cb1t = const.tile([128, CD], f32)
            cb2t = const.tile([128, CD], f32)
            b1ct = const.tile([128, JD], f32)
            if b2_nonzero:
                b2t = const.tile([128, D], f32)
            if ln1_affine:
                g1t = const.tile([128, D], f32)
                b1t = const.tile([128, D], f32)
            if ln2_affine:
                g2t = const.tile([128, D], f32)
                b2lt = const.tile([128, D], f32)

            def emit_consts():
                make_identity(nc, ident32)
                nc.vector.tensor_copy(out=identb[:], in_=ident32[:])
                nc.scalar.dma_start(out=cb1t[:], in_=cb1.rearrange("c p -> p c"))
                nc.scalar.dma_start(out=cb2t[:], in_=cb2.rearrange("c p -> p c"))
                nc.scalar.dma_start(out=b1ct[:], in_=b1c.rearrange("j p -> p j"))
                if b2_nonzero:
                    nc.scalar.dma_start(out=b2t[:], in_=bcast_row_ap(b2r, 0, 0, D))
                if ln1_affine:
                    nc.scalar.dma_start(out=g1t[:], in_=bcast_row_ap(g1r, 0, 0, D))
                    nc.scalar.dma_start(out=b1t[:], in_=bcast_row_ap(b1r, 0, 0, D))
                if ln2_affine:
                    nc.scalar.dma_start(out=g2t[:], in_=bcast_row_ap(g2r, 0, 0, D))
                    nc.scalar.dma_start(out=b2lt[:], in_=bcast_row_ap(b2lr, 0, 0, D))

            s2t = s2pool.tile([128, CD, PL], bf16)
            for c in range(CD):
                nc.vector.memset(s2t[:, c, 0:PAD], 0.0)
                nc.vector.memset(s2t[:, c, PAD + L:PL], 0.0)

            def masked_input(zpool, srcw, mt_cache, d, width=512, tag="zp"):
                """One tap: 4 c-chunk [128,width] bf16 rhs APs.  srcw(c) is
                the ALIGNED [128, width+2*PAD] source window starting at
                base-PAD; the masked product is computed over the whole
                window (mask-at-source) and the tap shift is applied by
                slicing the result at PAD+d."""
                if d == 0:
                    return [srcw(c)[:, PAD:PAD + width] for c in range(CD)]
                mt = mt_cache[d]
                zcs = []
                for c in range(CD):
                    zt = zpool.tile([128, width + 2 * PAD], bf16, tag=tag)
                    nc.vector.tensor_tensor(
                        out=zt[:], in0=srcw(c), in1=mt[:], op=MULT)
                    zcs.append(zt[:, PAD + d:PAD + d + width])
                return zcs

            def load_masks(mpool, taps, l0, eng, width=512, tag="maskp"):
                """masks is zero-padded by PAD on both sides; window
                [l0-PAD, l0+width+PAD) is padded cols [l0, l0+width+2PAD)."""
                mts = {}
                for (t, d) in taps:
                    if d == 0:
                        continue
                    mt = mpool.tile([128, width + 2 * PAD], bf16, tag=tag)
                    eng.dma_start(out=mt[:],
                                  in_=bcast_row_ap(masks, d2m[d], l0, width + 2 * PAD))
                    mts[d] = mt
                return mts

            # ================= stage 1 + stage 2 (channel-major) =================
            with (
                tc.tile_pool(name="s1p", bufs=1) as s1pool,
                tc.tile_pool(name="xin", bufs=16) as xin,
                tc.tile_pool(name="maskA", bufs=12) as mpoolA,
                tc.tile_pool(name="zpA", bufs=16) as zpoolA,
                tc.tile_pool(name="cps", bufs=8, space="PSUM") as cps,
            ):
                s1t = s1pool.tile([128, CD, PL], bf16)
                for c in range(CD):
                    nc.vector.memset(s1t[:, c, 0:PAD], 0.0)
                    nc.vector.memset(s1t[:, c, PAD + L:PL], 0.0)

                # deferred weight loads: stage-2 conv weights stream in
                # during stage-1 blocks; stage-3 conv + MLP weights during
                # stage-2 blocks.  Each stage's jobs are spread evenly over
                # its blocks, alternating the scalar/gpsimd queues.
                s2w0, s3w0 = 3 * CD * D, 8 * CD * D
                defer = [
                    col_slices(wct, wc, s2w0, s3w0, 16),
                    col_slices(wct, wc, s3w0, NT * CD * D, 16)
                    + [(lambda eng, c=c, h=h: eng.dma_start(
                        out=w1t[:, c, h * H // 2:(h + 1) * H // 2],
                        in_=w1[:, c, h * H // 2:(h + 1) * H // 2]))
                       for c in range(CD) for h in range(2)]
                    + [(lambda eng, jp=jp: eng.dma_start(
                        out=w2t[:, jp], in_=w2[:, jp]))
                       for jp in range(JD // 2)],
                ]

                def emit_defer(jobs, blk, nblk, engs):
                    lo = blk * len(jobs) // nblk
                    hi = (blk + 1) * len(jobs) // nblk
                    for j in range(lo, hi):
                        jobs[j](engs[j % len(engs)])

                for stage in (0, 1):
                    K = KS[stage]
                    taps = conv_taps[stage]
                    p = (K - 1) // 2
                    dstt, bct = (s1t, cb1t) if stage == 0 else (s2t, cb2t)
                    for blk in range(NBLK):
                        l0 = blk * 512
                        base = PAD + l0
                        if stage == 0:
                            xts = []
                            for c in range(CD):
                                xt = xin.tile([128, 512 + 2 * PAD], bf16, tag="xin")
                                nc.sync.dma_start(
                                    out=xt[:], in_=xcb[c, :, base - PAD:base + 512 + PAD])
                                xts.append(xt)
                            srcw = lambda c: xts[c][:]
                        else:
                            srcw = lambda c: s1t[:, c, base - PAD:base + 512 + PAD]
                        mts = load_masks(mpoolA, taps, l0, nc.gpsimd)
                        if stage == 0 and blk == 0:
                            emit_consts()
                        # stage-2 defers go sync-only: the gpsimd queue must
                        # stay clear so stage-3's first mask loads (gpsimd)
                        # aren't stuck behind megabytes of MLP weights at
                        # the stage-2 -> stage-3 transition.
                        emit_defer(defer[stage], blk, NBLK,
                                   (nc.sync, nc.gpsimd) if stage == 0 else (nc.sync,))
                        pss = [cps.tile([128, 512], f32, tag="cps", name=f"cps{o}") for o in range(CD)]
                        for ti, (t, d) in enumerate(taps):
                            zcs = masked_input(zpoolA, srcw, mts, d)
                            for c in range(CD):
                                for o in range(CD):
                                    a0 = (t * CD + c) * D + o * 128
                                    nc.tensor.matmul(
                                        pss[o][:],
                                        wct[:, a0:a0 + 128],
                                        zcs[c],
                                        start=(ti == 0 and c == 0),
                                        stop=(ti == K - 1 and c == CD - 1),
                                        skip_group_check=True,
                                    )
                        for o in range(CD):
                            nc.scalar.activation(
                                out=dstt[:, o, base:base + 512], in_=pss[o][:],
                                func=IDENT, bias=bct[:, o:o + 1], scale=1.0)

            # ============ stage 3 (L-major) + LN1 + MLP + LN2, fused ============
            with (
                tc.tile_pool(name="xbp", bufs=4) as xbp,
                tc.tile_pool(name="stp", bufs=4) as stp,
                tc.tile_pool(name="stats", bufs=10) as statp,
                tc.tile_pool(name="hbfp", bufs=10) as hbfp,
                tc.tile_pool(name="hctp", bufs=2) as hctp,
                tc.tile_pool(name="hidp", bufs=8) as hidp,
                tc.tile_pool(name="st2p", bufs=4) as st2p,
                tc.tile_pool(name="otp", bufs=4) as otp,
                tc.tile_pool(name="mask3", bufs=18) as mpoolB,
                tc.tile_pool(name="zp3", bufs=28) as zpoolB,
                tc.tile_pool(name="psum", bufs=5, space="PSUM") as psp,
            ):
                i32 = mybir.dt.int32
                SHR = mybir.AluOpType.logical_shift_right

                def rsqrt_var(v_ap, n=2):
                    """rs = 1/sqrt(v+eps) on DVE [128,n]: quake seed + 2
                    Newton.  One chain covers both 128-l chunks of a block
                    (cols), halving the serial-op count per LN."""
                    vt = statp.tile([128, n], f32, tag="vt")
                    nc.vector.tensor_scalar(
                        out=vt[:], in0=v_ap, scalar1=EPS, scalar2=None, op0=ADD)
                    y0b = statp.tile([128, n], i32, tag="y0b")
                    nc.vector.tensor_scalar(
                        out=y0b[:], in0=vt[:].bitcast(i32), scalar1=1,
                        scalar2=None, op0=SHR)
                    nc.vector.tensor_scalar(
                        out=y0b[:], in0=y0b[:], scalar1=-1, scalar2=0x5F3759DF,
                        op0=MULT, op1=ADD)
                    # one Newton step: quake seed ~3.4% -> ~0.17% rsqrt
                    # error, ~2e-3 on the output metric (budget 2e-2);
                    # the chain is latency-critical at every LN.
                    cur = y0b[:].bitcast(f32)
                    for it in range(1):
                        aq = statp.tile([128, n], f32, tag=f"nta{it}")
                        nc.vector.tensor_tensor(out=aq[:], in0=cur, in1=cur, op=MULT)
                        nc.vector.tensor_tensor(out=aq[:], in0=aq[:], in1=vt[:], op=MULT)
                        nc.vector.tensor_scalar(
                            out=aq[:], in0=aq[:], scalar1=-0.5, scalar2=1.5,
                            op0=MULT, op1=ADD)
                        yn = statp.tile([128, n], f32, tag=f"nty{it}")
                        nc.vector.tensor_tensor(out=yn[:], in0=cur, in1=aq[:], op=MULT)
                        cur = yn[:]
                    return cur

                def ln_stats(srcs):
                    """bn_stats/aggr over both chunks + one batched rsqrt.
                    Returns (mvall [128,4], rs [128,2])."""
                    mvall = statp.tile([128, 4], f32, tag="mva")
                    for i in range(2):
                        stats = statp.tile([128, 6], f32, tag="st6")
                        nc.vector.bn_stats(out=stats[:], in_=srcs[i][:])
                        nc.vector.bn_aggr(out=mvall[:, 2 * i:2 * i + 2], in_=stats[:])
                    rs = rsqrt_var(mvall[:, 1:4:2])
                    return mvall, rs

                taps = conv_taps[2]
                K = KS[2]
                NB2 = L // 256

                def conv3_mm(blk):
                    l0 = blk * 256
                    base = PAD + l0
                    srcw = lambda c: s2t[:, c, base - PAD:base + 256 + PAD]
                    mts = load_masks(mpoolB, taps, l0, nc.gpsimd, width=256, tag="mask3")
                    st3 = [psp.tile([128, 512], f32, tag="st3", bufs=4, name=f"st3_{i}") for i in range(2)]
                    for ti, (t, d) in enumerate(taps):
                        zcs = masked_input(zpoolB, srcw, mts, d, width=256, tag="zp3")
                        for c in range(CD):
                            a0 = (t * CD + c) * D
                            for i in range(2):
                                nc.tensor.matmul(
                                    st3[i][:],
                                    zcs[c][:, i * 128:(i + 1) * 128],
                                    wct[:, a0:a0 + D],
                                    start=(ti == 0 and c == 0),
                                    stop=(ti == K - 1 and c == CD - 1),
                                    skip_group_check=True,
                                )
                    return st3

                def drain3(blk, st3):
                    # residual add straight out of PSUM -> frees st3 banks
                    # early.  High priority: the scheduler otherwise orders
                    # this (and the LN1 chain it feeds) behind later z-mult
                    # batches, starving the PE's transpose of hb.
                    sts = []
                    with tc.high_priority():
                        for i in range(2):
                            lg = blk * 2 + i
                            xbt = xbp.tile([128, D], f32, tag="xbp")
                            nc.gpsimd.dma_start(out=xbt[:], in_=xb[lg])
                            st = stp.tile([128, D], f32, tag="stp")
                            nc.vector.scalar_tensor_tensor(
                                out=st[:], in0=st3[i][:], scalar=1.0, in1=xbt[:],
                                op0=MULT, op1=ADD)
                            sts.append(st)
                    return sts

                def post(blk, sts, pre, last=False):
                    # LN1; h kept bf16 (matmul + residual reuse).  The
                    # stats+rsqrt chain (pre) was computed a block ahead —
                    # its serial 10-op tail gets stretched by interleaved
                    # z-mults, so running it late would starve the PE's
                    # transpose of hb.  Only the two normalizes remain here.
                    mvall, rs = pre
                    with tc.high_priority():
                        hbfs = []
                        for i in range(2):
                            hb = hbfp.tile([128, D], bf16, tag="hbf")
                            nc.vector.tensor_scalar(
                                out=hb[:], in0=sts[i][:], scalar1=mvall[:, 2 * i:2 * i + 1],
                                scalar2=rs[:, i:i + 1], op0=SUB, op1=MULT)
                            if ln1_affine:
                                nc.vector.tensor_tensor(out=hb[:], in0=hb[:], in1=g1t[:], op=MULT)
                                nc.vector.tensor_tensor(out=hb[:], in0=hb[:], in1=b1t[:], op=ADD)
                            hbfs.append(hb)
                    # transpose h -> hct (channel-major, fp8) for mlp1; all
                    # four d-chunks packed into one PSUM bank.  The cast-copy
                    # runs on the Act engine so it never queues behind the
                    # Vector LN/drain backlog.
                    hct = hctp.tile([128, CD, 256], f8, tag="hct")
                    pt_all = psp.tile([128, CD, 256], bf16, tag="psA", bufs=2)
                    for i in range(2):
                        for dd in range(CD):
                            nc.tensor.transpose(
                                pt_all[:, dd, i * 128:(i + 1) * 128],
                                hbfs[i][:, dd * 128:(dd + 1) * 128],
                                identb[:],
                            )
                    with tc.high_priority():
                        nc.scalar.activation(out=hct[:], in_=pt_all[:], func=IDENT)
                    # mlp1/mlp2 in fp8 DoubleRow (two 128-row contraction
                    # chunks per matmul, 2x PE rate).  Weights are host-scaled
                    # by 64 so they sit in e4m3's normal range; the 1/64 is
                    # folded into the gelu activation scale (mlp1) and the
                    # LN2 residual drain (mlp2).
                    psB = [psp.tile([128, 512], f32, tag="psB", bufs=2, name=f"psB{i}") for i in range(2)]
                    for jp in range(JD // 2):
                        psa = psp.tile([128, 512], f32, tag="psA", bufs=2)
                        for jj in range(2):
                            j = jp * 2 + jj
                            for dp in range(2):
                                nc.tensor.matmul(
                                    psa[:, jj * 256:(jj + 1) * 256],
                                    w1t[:, 2 * dp:2 * dp + 2, j * 128:(j + 1) * 128],
                                    hct[:, 2 * dp:2 * dp + 2, :],
                                    start=(dp == 0),
                                    stop=(dp == 1),
                                    perf_mode=DR,
                                    skip_group_check=True,
                                )
                        hpair = hidp.tile([128, 2, 256], f8, tag="hid")
                        if b1_zero:
                            # mlp_b1 == 0: one gelu over the whole psa bank
                            # (hpair's [jj, l] free layout matches psa's)
                            nc.scalar.activation(
                                out=hpair[:], in_=psa[:],
                                func=GELU, scale=1.0 / 64.0)
                        else:
                            for jj in range(2):
                                j = jp * 2 + jj
                                nc.scalar.activation(
                                    out=hpair[:, jj], in_=psa[:, jj * 256:(jj + 1) * 256],
                                    func=GELU, bias=b1ct[:, j:j + 1], scale=1.0 / 64.0)
                        for i in range(2):
                            nc.tensor.matmul(
                                psB[i][:],
                                hpair[:, :, i * 128:(i + 1) * 128],
                                w2t[:, jp],
                                start=(jp == 0),
                                stop=(jp == JD // 2 - 1),
                                perf_mode=DR,
                                skip_group_check=True,
                            )
                    # LN2 per 128-l chunk, straight from PSUM (1/64 undoes
                    # the fp8 w2 host-scale)
                    def ln2_chunk(i, mvc, rsc):
                        ot = otp.tile([128, D], f32, tag="otp")
                        nc.vector.tensor_scalar(
                            out=ot[:], in0=st2s[i][:], scalar1=mvc,
                            scalar2=rsc, op0=SUB, op1=MULT)
                        if ln2_affine:
                            nc.vector.tensor_tensor(out=ot[:], in0=ot[:], in1=g2t[:], op=MULT)
                            nc.vector.tensor_tensor(out=ot[:], in0=ot[:], in1=b2lt[:], op=ADD)
                        lr = (blk * 2 + i) * 128
                        # chunk 0 on gpsimd, chunk 1 on sync: the final two
                        # stores overlap instead of serializing at the tail
                        eng = nc.gpsimd if i == 0 else nc.sync
                        eng.dma_start(out=out[lr:lr + 128, :], in_=ot[:])

                    def st2_chunk(i):
                        st2 = st2p.tile([128, D], f32, tag="st2")
                        nc.vector.scalar_tensor_tensor(
                            out=st2[:], in0=psB[i][:], scalar=1.0 / 64.0, in1=hbfs[i][:],
                            op0=MULT, op1=ADD)
                        if b2_nonzero:
                            nc.vector.tensor_tensor(out=st2[:], in0=st2[:], in1=b2t[:], op=ADD)
                        return st2

                    if last:
                        # per-chunk chains: chunk 0's normalize + store
                        # overlap chunk 1's matmuls/stats at the kernel tail
                        st2s = []
                        for i in range(2):
                            st2s.append(st2_chunk(i))
                            stats = statp.tile([128, 6], f32, tag="st6")
                            nc.vector.bn_stats(out=stats[:], in_=st2s[i][:])
                            mv = statp.tile([128, 2], f32, tag="mva")
                            nc.vector.bn_aggr(out=mv[:], in_=stats[:])
                            rs = rsqrt_var(mv[:, 1:2], n=1)
                            ln2_chunk(i, mv[:, 0:1], rs[:, 0:1])
                    else:
                        st2s = [st2_chunk(0), st2_chunk(1)]
                        mvall2, rs2 = ln_stats(st2s)
                        for i in range(2):
                            ln2_chunk(i, mvall2[:, 2 * i:2 * i + 1], rs2[:, i:i + 1])

                # 2-deep software pipeline: conv matmuls run two blocks
                # ahead of the LN/MLP stage so the PE never waits on the
                # Vector LN1 chain (st3 ring=4 holds two blocks' banks).
                # drain3(b+2) is emitted AFTER post(b): the Vector queue is
                # in-order, and drain3(b+2) blocks on conv3(b+2)'s last
                # matmul — emitting it earlier would stall LN1(b) (and with
                # it the PE's transpose+mlp1) behind the conv matmul batch.
                sts_q = [drain3(0, conv3_mm(0)), drain3(1, conv3_mm(1))]
                with tc.high_priority():
                    pre_q = [ln_stats(sts_q[0])]
                for blk in range(NB2):
                    st3n = conv3_mm(blk + 2) if blk + 2 < NB2 else None
                    if blk + 1 < NB2:
                        with tc.high_priority():
                            pre_q.append(ln_stats(sts_q[blk + 1]))
                    post(blk, sts_q[blk], pre_q[blk])
                    if st3n is not None:
                        sts_q.append(drain3(blk + 2, st3n))

    nc.compile()
    return nc


def _prep_inputs(x, chain, W3, b3, W5, b5, W7, b7,
                 mlp_w1, mlp_b1, mlp_w2, mlp_b2,
                 ln1_g, ln1_b, ln2_g, ln2_b):
    import ml_dtypes

    f32 = np.float32
    bf = ml_dtypes.bfloat16
    x = np.asarray(x, f32)
    chain = np.asarray(chain, np.int32)
    flags = (
        not (np.all(np.asarray(ln1_g) == 1.0) and np.all(np.asarray(ln1_b) == 0.0)),
        not (np.all(np.asarray(ln2_g) == 1.0) and np.all(np.asarray(ln2_b) == 0.0)),
        bool(np.any(np.asarray(mlp_b2) != 0.0)),
        not np.any(np.asarray(mlp_b1) != 0.0),
    )

    # conv weights: per global tap t -> W[:, :, kt].T  (shape [c, o])
    wct = np.empty((NT, D, D), f32)
    t = 0
    for W in (W3, W5, W7):
        W = np.asarray(W, f32)
        for k in range(W.shape[2]):
            wct[t] = W[:, :, k].T
            t += 1
    # partition-major flat: wc[p, ((t*CD + c)*D + o)] = W_t[c*128+p, o]
    wc = np.ascontiguousarray(
        wct.reshape(NT, CD, 128, D).transpose(2, 0, 1, 3).reshape(128, NT * CD * D)
    ).astype(bf)

    f8 = ml_dtypes.float8_e4m3
    shared = {
        "wc": wc,
        "cb1": np.asarray(b3, f32).reshape(CD, 128),
        "cb2": np.asarray(b5, f32).reshape(CD, 128),
        # MLP weights in fp8 e4m3 (DoubleRow matmuls), host-scaled by 64
        # into e4m3's normal range; the kernel folds 1/64 back in.
        "w1": np.ascontiguousarray(
            np.asarray(mlp_w1, f32).reshape(CD, 128, H).transpose(1, 0, 2)
            * 64.0).astype(f8),
        "b1c": np.asarray(mlp_b1, f32).reshape(JD, 128),
        "w2": np.ascontiguousarray(
            np.asarray(mlp_w2, f32).reshape(JD, 128, D).transpose(1, 0, 2)
            .reshape(128, JD // 2, 2, D) * 64.0).astype(f8),
    }
    if flags[0]:
        shared["g1r"] = np.asarray(ln1_g, f32).reshape(1, D)
        shared["b1r"] = np.asarray(ln1_b, f32).reshape(1, D)
    if flags[1]:
        shared["g2r"] = np.asarray(ln2_g, f32).reshape(1, D)
        shared["b2lr"] = np.asarray(ln2_b, f32).reshape(1, D)
    if flags[2]:
        shared["b2r"] = np.asarray(mlp_b2, f32).reshape(1, D)

    b7f = np.asarray(b7, f32)
    in_maps = []
    for b in range(B):
        xc = x[b].T  # (D, L)
        xcp = np.zeros((CD, 128, PL), f32)
        xcp[:, :, PAD:PAD + L] = xc.reshape(CD, 128, L)
        xbv = (x[b] + b7f[None, :]).reshape(LCH, 128, D)
        # masks for shifts d in (-3,-2,-1,1,2,3), evaluated at output position
        ce = np.zeros(L + 8, np.int32)
        ce[4:4 + L] = chain[b]
        # masks are zero-padded by PAD cols each side (kernel reads aligned
        # [l0-PAD, l0+width+PAD) windows); row order is shift (-3..-1,1..3)
        m = np.zeros((6, PAD + L + PAD), bf)
        for mi, d in enumerate((-3, -2, -1, 1, 2, 3)):
            m[mi, PAD:PAD + L] = (ce[4 + d:4 + d + L] == chain[b]).astype(bf)
        im = {"xcb": xcp.astype(bf), "xb": np.ascontiguousarray(xbv),
              "masks": m, **shared}
        in_maps.append(im)
    return in_maps, flags


def kernel(**inputs):
    from concourse.bass_utils import run_bass_kernel_spmd

    in_maps, flags = _prep_inputs(**inputs)
    if flags not in _CACHE:
        _CACHE[flags] = _build_nc(*flags)
    nc = _CACHE[flags]
    res = run_bass_kernel_spmd(nc, in_maps, list(range(NCORES)))
    return np.stack([res.results[b]["out"] for b in range(B)]).astype(np.float32)



# revision 84
# speedup vs baseline: 1.0118x; 1.0118x over previous
"""ConvFormer Trainium2 kernel v3 — fused single-pass; bf16 convs, fp8 MLP.

Data-parallel over B across 8 NeuronCores (batch element b -> core b).

Per core:
    x1 = x.T (channel-major); 3 chained masked convs (K=3,5,7) where each
    tap's contribution is masked by (chain[l+d]==chain[l]); h = LN(x + x1.T);
    out = LN(h + gelu(h@w1+b1)@w2 + b2).

v3 design (vs v2 all-bf16 at ~802 us; v3 measures ~584 us):
  - MLP in fp8 e4m3 DoubleRow (2 contraction chunks/instr, 2x PE rate);
    weights host-scaled x64 into e4m3's normal range, 1/64 folded into the
    gelu scale (mlp1) and LN2 residual drain (mlp2).  Convs must stay bf16:
    even one conv stage in fp8 measures ~3e-2 > the 2e-2 budget (conv path
    has ~unit weight in the output; the MLP path is ~0.2).
  - conv stages SBUF-resident; stage 3 L-major straight into the LN layout;
    MLP fused per 256-token block, 2-deep software pipeline (conv3 runs two
    blocks ahead of LN/MLP; PSUM: st3 ring 4 + psa/pt 2 + psB 2 = 8 banks).
  - LN1 stats+rsqrt computed one block AHEAD at scheduler high priority:
    the serial quake+Newton chain gets stretched ~3x by interleaved z-mults
    and otherwise starves the PE's h-transpose.  One Newton step (~0.17%
    rsqrt error, ~2e-3 on the metric).
  - DMA discipline: dma_start blocks its issuing queue on the hw-DMA-ring
    semaphore, so the Act queue carries no bulk DMAs (they'd head-of-line
    block the conv-stage activations); stage-1 weights land tap-by-tap
    before anything else; stage-2/3 conv + MLP weights stream in lazily
    during earlier stages on the sync/gpsimd queues.
  - z-mults multiply unshifted source windows by the mask-at-source (tap
    shift folded into the matmul read AP), keeping DVE reads aligned.
"""

import numpy as np

B, L, D = 8, 4096, 512
KS = (3, 5, 7)
EPS = 1e-5
NCORES = 8
PAD = 4            # zero-pad cols each side of channel-major tensors
PL = PAD + L + PAD
NBLK = L // 512    # 8 blocks of 512
CD = D // 128      # 4 channel chunks
LCH = L // 128     # 32 L-chunks of 128
H = 4 * D          # mlp hidden
JD = H // 128      # 16 hidden chunks
NT = sum(KS)       # 15 taps total
GELU_FUNC_NAME = "Gelu_apprx_tanh"  # jax.nn.gelu default is approximate=True

_CACHE = {}


def _build_nc(ln1_affine, ln2_affine, b2_nonzero, b1_zero):
    import concourse.bass as bass
    import concourse.tile as tile
    from concourse import bacc, mybir
    from concourse.masks import make_identity

    f32 = mybir.dt.float32
    bf16 = mybir.dt.bfloat16
    f8 = mybir.dt.float8e4
    DR = mybir.MatmulPerfMode.DoubleRow
    GELU = getattr(mybir.ActivationFunctionType, GELU_FUNC_NAME)
    IDENT = mybir.ActivationFunctionType.Identity
    ADD = mybir.AluOpType.add
    SUB = mybir.AluOpType.subtract
    MULT = mybir.AluOpType.mult
    POW = mybir.AluOpType.pow

    # global tap -> (conv index, shift d); convs use taps [0:3], [3:8], [8:15]
    conv_taps = []
    t0 = 0
    for K in KS:
        p = (K - 1) // 2
        conv_taps.append([(t0 + i, i - p) for i in range(K)])
        t0 += K
    # z-mults multiply UNSHIFTED source windows by the mask-at-source
    # (so the DVE reads stay 2-element aligned and hit the 16-bit 2X
    # perf mode); the tap shift d moves into the matmul's read AP.
    # mask-at-source for tap d is the host mask row for shift -d.
    # host mask rows are ordered by shift (-3,-2,-1,1,2,3).
    d2m = {-3: 5, -2: 4, -1: 3, 1: 2, 2: 1, 3: 0}

    nc = bacc.Bacc(None, target_bir_lowering=False)

    xcb = nc.declare_dram_parameter("xcb", [CD, 128, PL], bf16, isOutput=False)
    xb = nc.declare_dram_parameter("xb", [LCH, 128, D], f32, isOutput=False)
    masks = nc.declare_dram_parameter("masks", [6, PAD + L + PAD], bf16, isOutput=False)
    wc = nc.declare_dram_parameter("wc", [128, NT * CD * D], bf16, isOutput=False)
    cb1 = nc.declare_dram_parameter("cb1", [CD, 128], f32, isOutput=False)
    cb2 = nc.declare_dram_parameter("cb2", [CD, 128], f32, isOutput=False)
    w1 = nc.declare_dram_parameter("w1", [128, CD, H], f8, isOutput=False)
    b1c = nc.declare_dram_parameter("b1c", [JD, 128], f32, isOutput=False)
    w2 = nc.declare_dram_parameter("w2", [128, JD // 2, 2, D], f8, isOutput=False)
    if b2_nonzero:
        b2r = nc.declare_dram_parameter("b2r", [1, D], f32, isOutput=False)
    if ln1_affine:
        g1r = nc.declare_dram_parameter("g1r", [1, D], f32, isOutput=False)
        b1r = nc.declare_dram_parameter("b1r", [1, D], f32, isOutput=False)
    if ln2_affine:
        g2r = nc.declare_dram_parameter("g2r", [1, D], f32, isOutput=False)
        b2lr = nc.declare_dram_parameter("b2lr", [1, D], f32, isOutput=False)
    out = nc.declare_dram_parameter("out", [L, D], f32, isOutput=True)

    def bcast_row_ap(param, row, col0, n):
        """DMA access pattern: one DRAM row slice broadcast to 128 partitions."""
        src = param[row, col0:col0 + n]
        return bass.AP(tensor=src.tensor, offset=src.offset, ap=[[0, 128]] + list(src.ap))

    with tile.TileContext(nc) as tc:
        with (
            tc.tile_pool(name="const", bufs=1) as const,
            tc.tile_pool(name="wcp", bufs=1) as wcp,
            tc.tile_pool(name="wm", bufs=1) as wm,
            tc.tile_pool(name="s2p", bufs=1) as s2pool,
        ):
            wct = wcp.tile([128, NT * CD * D], bf16)
            w1t = wm.tile([128, CD, H], f8)
            w2t = wm.tile([128, JD // 2, 2, D], f8)

            # stage-1 weights FIRST, tap by tap (the PE consumes them in
            # tap order, ~3.4us apart); tap 0 is split scalar/sync so the
            # first matmul's weights land fastest.  After the consts the
            # scalar queue carries NO DMAs: a queued dma_start blocks on
            # the queue's hw-DMA-ring semaphore and would head-of-line
            # block the conv-stage activations sharing the Act engine.
            # Deferred weight loads go on sync+gpsimd instead.
            half = CD * D // 2
            nc.scalar.dma_start(out=wct[:, 0:half], in_=wc[:, 0:half])
            nc.gpsimd.dma_start(out=wct[:, half:CD * D], in_=wc[:, half:CD * D])
            for t in (1, 2):
                for c in range(CD):
                    a0 = (t * CD + c) * D
                    nc.scalar.dma_start(out=wct[:, a0:a0 + D], in_=wc[:, a0:a0 + D])

            # deferred-load helper: large tensors are emitted in slices,
            # a few per (stage, block) iteration, alternating queues.
            def col_slices(dst, src, c0, c1, nparts):
                n = c1 - c0
                step = -(-n // nparts)
                return [
                    (lambda eng, a=c0 + s, b=min(c0 + s + step, c1):
                     eng.dma_start(out=dst[:, a:b], in_=src[:, a:b]))
                    for s in range(0, n, step)
                ]

            ident32 = const.tile([128, 128], f32)
            identb = const.tile([128, 128], bf16)
            cb1t = const.tile([128, CD], f32)
            cb2t = const.tile([128, CD], f32)
            b1ct = const.tile([128, JD], f32)
            if b2_nonzero:
                b2t = const.tile([128, D], f32)
            if ln1_affine:
                g1t = const.tile([128, D], f32)
                b1t = const.tile([128, D], f32)
            if ln2_affine:
                g2t = const.tile([128, D], f32)
                b2lt = const.tile([128, D], f32)

            def emit_consts():
                make_identity(nc, ident32)
                nc.vector.tensor_copy(out=identb[:], in_=ident32[:])
                nc.scalar.dma_start(out=cb1t[:], in_=cb1.rearrange("c p -> p c"))
                nc.scalar.dma_start(out=cb2t[:], in_=cb2.rearrange("c p -> p c"))
                nc.scalar.dma_start(out=b1ct[:], in_=b1c.rearrange("j p -> p j"))
                if b2_nonzero:
                    nc.scalar.dma_start(out=b2t[:], in_=bcast_row_ap(b2r, 0, 0, D))
                if ln1_affine:
                    nc.scalar.dma_start(out=g1t[:], in_=bcast_row_ap(g1r, 0, 0, D))
                    nc.scalar.dma_start(out=b1t[:], in_=bcast_row_ap(b1r, 0, 0, D))
                if ln2_affine:
                    nc.scalar.dma_start(out=g2t[:], in_=bcast_row_ap(g2r, 0, 0, D))
                    nc.scalar.dma_start(out=b2lt[:], in_=bcast_row_ap(b2lr, 0, 0, D))

            s2t = s2pool.tile([128, CD, PL], bf16)
            for c in range(CD):
                nc.vector.memset(s2t[:, c, 0:PAD], 0.0)
                nc.vector.memset(s2t[:, c, PAD + L:PL], 0.0)

            def masked_input(zpool, srcw, mt_cache, d, width=512, tag="zp"):
                """One tap: 4 c-chunk [128,width] bf16 rhs APs.  srcw(c) is
                the ALIGNED [128, width+2*PAD] source window starting at
                base-PAD; the masked product is computed over the whole
                window (mask-at-source) and the tap shift is applied by
                slicing the result at PAD+d."""
                if d == 0:
                    return [srcw(c)[:, PAD:PAD + width] for c in range(CD)]
                mt = mt_cache[d]
                zcs = []
                for c in range(CD):
                    zt = zpool.tile([128, width + 2 * PAD], bf16, tag=tag)
                    nc.vector.tensor_tensor(
                        out=zt[:], in0=srcw(c), in1=mt[:], op=MULT)
                    zcs.append(zt[:, PAD + d:PAD + d + width])
                return zcs

            def load_masks(mpool, taps, l0, eng, width=512, tag="maskp"):
                """masks is zero-padded by PAD on both sides; window
                [l0-PAD, l0+width+PAD) is padded cols [l0, l0+width+2PAD)."""
                mts = {}
                for (t, d) in taps:
                    if d == 0:
                        continue
                    mt = mpool.tile([128, width + 2 * PAD], bf16, tag=tag)
                    eng.dma_start(out=mt[:],
                                  in_=bcast_row_ap(masks, d2m[d], l0, width + 2 * PAD))
                    mts[d] = mt
                return mts

            # ================= stage 1 + stage 2 (channel-major) =================
            with (
                tc.tile_pool(name="s1p", bufs=1) as s1pool,
                tc.tile_pool(name="xin", bufs=16) as xin,
                tc.tile_pool(name="maskA", bufs=12) as mpoolA,
                tc.tile_pool(name="zpA", bufs=16) as zpoolA,
                tc.tile_pool(name="cps", bufs=8, space="PSUM") as cps,
            ):
                s1t = s1pool.tile([128, CD, PL], bf16)
                for c in range(CD):
                    nc.vector.memset(s1t[:, c, 0:PAD], 0.0)
                    nc.vector.memset(s1t[:, c, PAD + L:PL], 0.0)

                # deferred weight loads: stage-2 conv weights stream in
                # during stage-1 blocks; stage-3 conv + MLP weights during
                # stage-2 blocks.  Each stage's jobs are spread evenly over
                # its blocks, alternating the scalar/gpsimd queues.
                s2w0, s3w0 = 3 * CD * D, 8 * CD * D
                defer = [
                    col_slices(wct, wc, s2w0, s3w0, 16),
                    col_slices(wct, wc, s3w0, NT * CD * D, 16)
                    + [(lambda eng, c=c, h=h: eng.dma_start(
                        out=w1t[:, c, h * H // 2:(h + 1) * H // 2],
                        in_=w1[:, c, h * H // 2:(h + 1) * H // 2]))
                       for c in range(CD) for h in range(2)]
                    + [(lambda eng, jp=jp: eng.dma_start(
                        out=w2t[:, jp], in_=w2[:, jp]))
                       for jp in range(JD // 2)],
                ]

                def emit_defer(jobs, blk, nblk, engs):
                    lo = blk * len(jobs) // nblk
                    hi = (blk + 1) * len(jobs) // nblk
                    for j in range(lo, hi):
                        jobs[j](engs[j % len(engs)])

                for stage in (0, 1):
                    K = KS[stage]
                    taps = conv_taps[stage]
                    p = (K - 1) // 2
                    dstt, bct = (s1t, cb1t) if stage == 0 else (s2t, cb2t)
                    for blk in range(NBLK):
                        l0 = blk * 512
                        base = PAD + l0
                        if stage == 0:
                            xts = []
                            for c in range(CD):
                                xt = xin.tile([128, 512 + 2 * PAD], bf16, tag="xin")
                                nc.sync.dma_start(
                                    out=xt[:], in_=xcb[c, :, base - PAD:base + 512 + PAD])
                                xts.append(xt)
                            srcw = lambda c: xts[c][:]
                        else:
                            srcw = lambda c: s1t[:, c, base - PAD:base + 512 + PAD]
                        mts = load_masks(mpoolA, taps, l0, nc.gpsimd)
                        if stage == 0 and blk == 0:
                            emit_consts()
                        # stage-2 defers go sync-only: the gpsimd queue must
                        # stay clear so stage-3's first mask loads (gpsimd)
                        # aren't stuck behind megabytes of MLP weights at
                        # the stage-2 -> stage-3 transition.
                        emit_defer(defer[stage], blk, NBLK,
                                   (nc.sync, nc.gpsimd) if stage == 0 else (nc.sync,))
                        pss = [cps.tile([128, 512], f32, tag="cps", name=f"cps{o}") for o in range(CD)]
                        for ti, (t, d) in enumerate(taps):
                            zcs = masked_input(zpoolA, srcw, mts, d)
                            for c in range(CD):
                                for o in range(CD):
                                    a0 = (t * CD + c) * D + o * 128
                                    nc.tensor.matmul(
                                        pss[o][:],
                                        wct[:, a0:a0 + 128],
                                        zcs[c],
                                        start=(ti == 0 and c == 0),
                                        stop=(ti == K - 1 and c == CD - 1),
                                        skip_group_check=True,
                                    )
                        for o in range(CD):
                            nc.scalar.activation(
                                out=dstt[:, o, base:base + 512], in_=pss[o][:],
                                func=IDENT, bias=bct[:, o:o + 1], scale=1.0)

            # ============ stage 3 (L-major) + LN1 + MLP + LN2, fused ============
            with (
                tc.tile_pool(name="xbp", bufs=4) as xbp,
                tc.tile_pool(name="stp", bufs=4) as stp,
                tc.tile_pool(name="stats", bufs=10) as statp,
                tc.tile_pool(name="hbfp", bufs=10) as hbfp,
                tc.tile_pool(name="hctp", bufs=2) as hctp,
                tc.tile_pool(name="hidp", bufs=8) as hidp,
                tc.tile_pool(name="st2p", bufs=4) as st2p,
                tc.tile_pool(name="otp", bufs=4) as otp,
                tc.tile_pool(name="mask3", bufs=18) as mpoolB,
                tc.tile_pool(name="zp3", bufs=28) as zpoolB,
                tc.tile_pool(name="psum", bufs=5, space="PSUM") as psp,
            ):
                i32 = mybir.dt.int32
                SHR = mybir.AluOpType.logical_shift_right

                def rsqrt_var(v_ap, n=2):
                    """rs = 1/sqrt(v+eps) on DVE [128,n]: quake seed + 2
                    Newton.  One chain covers both 128-l chunks of a block
                    (cols), halving the serial-op count per LN."""
                    vt = statp.tile([128, n], f32, tag="vt")
                    nc.vector.tensor_scalar(
                        out=vt[:], in0=v_ap, scalar1=EPS, scalar2=None, op0=ADD)
                    y0b = statp.tile([128, n], i32, tag="y0b")
                    nc.vector.tensor_scalar(
                        out=y0b[:], in0=vt[:].bitcast(i32), scalar1=1,
                        scalar2=None, op0=SHR)
                    nc.vector.tensor_scalar(
                        out=y0b[:], in0=y0b[:], scalar1=-1, scalar2=0x5F3759DF,
                        op0=MULT, op1=ADD)
                    # one Newton step: quake seed ~3.4% -> ~0.17% rsqrt
                    # error, ~2e-3 on the output metric (budget 2e-2);
                    # the chain is latency-critical at every LN.
                    cur = y0b[:].bitcast(f32)
                    for it in range(1):
                        aq = statp.tile([128, n], f32, tag=f"nta{it}")
                        nc.vector.tensor_tensor(out=aq[:], in0=cur, in1=cur, op=MULT)
                        nc.vector.tensor_tensor(out=aq[:], in0=aq[:], in1=vt[:], op=MULT)
                        nc.vector.tensor_scalar(
                            out=aq[:], in0=aq[:], scalar1=-0.5, scalar2=1.5,
                            op0=MULT, op1=ADD)
                        yn = statp.tile([128, n], f32, tag=f"nty{it}")
                        nc.vector.tensor_tensor(out=yn[:], in0=cur, in1=aq[:], op=MULT)
                        cur = yn[:]
                    return cur

                def ln_stats(srcs):
                    """bn_stats/aggr over both chunks + one batched rsqrt.
                    Returns (mvall [128,4], rs [128,2])."""
                    mvall = statp.tile([128, 4], f32, tag="mva")
                    for i in range(2):
                        stats = statp.tile([128, 6], f32, tag="st6")
                        nc.vector.bn_stats(out=stats[:], in_=srcs[i][:])
                        nc.vector.bn_aggr(out=mvall[:, 2 * i:2 * i + 2], in_=stats[:])
                    rs = rsqrt_var(mvall[:, 1:4:2])
                    return mvall, rs

                taps = conv_taps[2]
                K = KS[2]
                NB2 = L // 256

                def conv3_mm(blk):
                    l0 = blk * 256
                    base = PAD + l0
                    srcw = lambda c: s2t[:, c, base - PAD:base + 256 + PAD]
                    mts = load_masks(mpoolB, taps, l0, nc.gpsimd, width=256, tag="mask3")
                    st3 = [psp.tile([128, 512], f32, tag="st3", bufs=4, name=f"st3_{i}") for i in range(2)]
                    for ti, (t, d) in enumerate(taps):
                        zcs = masked_input(zpoolB, srcw, mts, d, width=256, tag="zp3")
                        for c in range(CD):
                            a0 = (t * CD + c) * D
                            for i in range(2):
                                nc.tensor.matmul(
                                    st3[i][:],
                                    zcs[c][:, i * 128:(i + 1) * 128],
                                    wct[:, a0:a0 + D],
                                    start=(ti == 0 and c == 0),
                                    stop=(ti == K - 1 and c == CD - 1),
                                    skip_group_check=True,
                                )
                    return st3

                def drain3(blk, st3):
                    # residual add straight out of PSUM -> frees st3 banks
                    # early.  High priority: the scheduler otherwise orders
                    # this (and the LN1 chain it feeds) behind later z-mult
                    # batches, starving the PE's transpose of hb.
                    sts = []
                    with tc.high_priority():
                        for i in range(2):
                            lg = blk * 2 + i
                            xbt = xbp.tile([128, D], f32, tag="xbp")
                            nc.gpsimd.dma_start(out=xbt[:], in_=xb[lg])
                            st = stp.tile([128, D], f32, tag="stp")
                            nc.vector.scalar_tensor_tensor(
                                out=st[:], in0=st3[i][:], scalar=1.0, in1=xbt[:],
                                op0=MULT, op1=ADD)
                            sts.append(st)
                    return sts

                def post(blk, sts, pre, last=False):
                    # LN1; h kept bf16 (matmul + residual reuse).  The
                    # stats+rsqrt chain (pre) was computed a block ahead —
                    # its serial 10-op tail gets stretched by interleaved
                    # z-mults, so running it late would starve the PE's
                    # transpose of hb.  Only the two normalizes remain here.
                    mvall, rs = pre
                    with tc.high_priority():
                        hbfs = []
                        for i in range(2):
                            hb = hbfp.tile([128, D], bf16, tag="hbf")
                            nc.vector.tensor_scalar(
                                out=hb[:], in0=sts[i][:], scalar1=mvall[:, 2 * i:2 * i + 1],
                                scalar2=rs[:, i:i + 1], op0=SUB, op1=MULT)
                            if ln1_affine:
                                nc.vector.tensor_tensor(out=hb[:], in0=hb[:], in1=g1t[:], op=MULT)
                                nc.vector.tensor_tensor(out=hb[:], in0=hb[:], in1=b1t[:], op=ADD)
                            hbfs.append(hb)
                    # transpose h -> hct (channel-major, fp8) for mlp1; all
                    # four d-chunks packed into one PSUM bank.  The cast-copy
                    # runs on the Act engine so it never queues behind the
                    # Vector LN/drain backlog.
                    hct = hctp.tile([128, CD, 256], f8, tag="hct")
                    pt_all = psp.tile([128, CD, 256], bf16, tag="psA", bufs=2)
                    for i in range(2):
                        for dd in range(CD):
                            nc.tensor.transpose(
                                pt_all[:, dd, i * 128:(i + 1) * 128],
                                hbfs[i][:, dd * 128:(dd + 1) * 128],
                                identb[:],
                            )
                    with tc.high_priority():
                        nc.scalar.activation(out=hct[:], in_=pt_all[:], func=IDENT)
                    # mlp1/mlp2 in fp8 DoubleRow (two 128-row contraction
                    # chunks per matmul, 2x PE rate).  Weights are host-scaled
                    # by 64 so they sit in e4m3's normal range; the 1/64 is
                    # folded into the gelu activation scale (mlp1) and the
                    # LN2 residual drain (mlp2).
                    psB = [psp.tile([128, 512], f32, tag="psB", bufs=2, name=f"psB{i}") for i in range(2)]
                    for jp in range(JD // 2):
                        psa = psp.tile([128, 512], f32, tag="psA", bufs=2)
                        for jj in range(2):
                            j = jp * 2 + jj
                            for dp in range(2):
                                nc.tensor.matmul(
                                    psa[:, jj * 256:(jj + 1) * 256],
                                    w1t[:, 2 * dp:2 * dp + 2, j * 128:(j + 1) * 128],
                                    hct[:, 2 * dp:2 * dp + 2, :],
                                    start=(dp == 0),
                                    stop=(dp == 1),
                                    perf_mode=DR,
                                    skip_group_check=True,
                                )
                        hpair = hidp.tile([128, 2, 256], f8, tag="hid")
                        if b1_zero:
                            # mlp_b1 == 0: one gelu over the whole psa bank
                            # (hpair's [jj, l] free layout matches psa's)
                            nc.scalar.activation(
                                out=hpair[:], in_=psa[:],
                                func=GELU, scale=1.0 / 64.0)
                        else:
                            for jj in range(2):
                                j = jp * 2 + jj
                                nc.scalar.activation(
                                    out=hpair[:, jj], in_=psa[:, jj * 256:(jj + 1) * 256],
                                    func=GELU, bias=b1ct[:, j:j + 1], scale=1.0 / 64.0)
                        for i in range(2):
                            nc.tensor.matmul(
                                psB[i][:],
                                hpair[:, :, i * 128:(i + 1) * 128],
                                w2t[:, jp],
                                start=(jp == 0),
                                stop=(jp == JD // 2 - 1),
                                perf_mode=DR,
                                skip_group_check=True,
                            )
                    # LN2 per 128-l chunk, straight from PSUM (1/64 undoes
                    # the fp8 w2 host-scale)
                    def ln2_chunk(i, mvc, rsc):
                        ot = otp.tile([128, D], f32, tag="otp")
                        nc.vector.tensor_scalar(
                            out=ot[:], in0=st2s[i][:], scalar1=mvc,
                            scalar2=rsc, op0=SUB, op1=MULT)
                        if ln2_affine:
                            nc.vector.tensor_tensor(out=ot[:], in0=ot[:], in1=g2t[:], op=MULT)
                            nc.vector.tensor_tensor(out=ot[:], in0=ot[:], in1=b2lt[:], op=ADD)
                        lr = (blk * 2 + i) * 128
                        nc.sync.dma_start(out=out[lr:lr + 128, :], in_=ot[:])

                    def st2_chunk(i):
                        st2 = st2p.tile([128, D], f32, tag="st2")
                        nc.vector.scalar_tensor_tensor(
                            out=st2[:], in0=psB[i][:], scalar=1.0 / 64.0, in1=hbfs[i][:],
                            op0=MULT, op1=ADD)
                        if b2_nonzero:
                            nc.vector.tensor_tensor(out=st2[:], in0=st2[:], in1=b2t[:], op=ADD)
                        return st2

                    if last:
                        # per-chunk chains: chunk 0's normalize + store
                        # overlap chunk 1's matmuls/stats at the kernel tail
                        st2s = []
                        for i in range(2):
                            st2s.append(st2_chunk(i))
                            stats = statp.tile([128, 6], f32, tag="st6")
                            nc.vector.bn_stats(out=stats[:], in_=st2s[i][:])
                            mv = statp.tile([128, 2], f32, tag="mva")
                            nc.vector.bn_aggr(out=mv[:], in_=stats[:])
                            rs = rsqrt_var(mv[:, 1:2], n=1)
                            ln2_chunk(i, mv[:, 0:1], rs[:, 0:1])
                    else:
                        st2s = [st2_chunk(0), st2_chunk(1)]
                        mvall2, rs2 = ln_stats(st2s)
                        for i in range(2):
                            ln2_chunk(i, mvall2[:, 2 * i:2 * i + 1], rs2[:, i:i + 1])

                # 2-deep software pipeline: conv matmuls run two blocks
                # ahead of the LN/MLP stage so the PE never waits on the
                # Vector LN1 chain (st3 ring=4 holds two blocks' banks).
                # drain3(b+2) is emitted AFTER post(b): the Vector queue is
                # in-order, and drain3(b+2) blocks on conv3(b+2)'s last
                # matmul — emitting it earlier would stall LN1(b) (and with
                # it the PE's transpose+mlp1) behind the conv matmul batch.
                sts_q = [drain3(0, conv3_mm(0)), drain3(1, conv3_mm(1))]
                with tc.high_priority():
                    pre_q = [ln_stats(sts_q[0])]
                for blk in range(NB2):
                    st3n = conv3_mm(blk + 2) if blk + 2 < NB2 else None
                    if blk + 1 < NB2:
                        with tc.high_priority():
                            pre_q.append(ln_stats(sts_q[blk + 1]))
                    post(blk, sts_q[blk], pre_q[blk])
                    if st3n is not None:
                        sts_q.append(drain3(blk + 2, st3n))

    nc.compile()
    return nc


def _prep_inputs(x, chain, W3, b3, W5, b5, W7, b7,
                 mlp_w1, mlp_b1, mlp_w2, mlp_b2,
                 ln1_g, ln1_b, ln2_g, ln2_b):
    import ml_dtypes

    f32 = np.float32
    bf = ml_dtypes.bfloat16
    x = np.asarray(x, f32)
    chain = np.asarray(chain, np.int32)
    flags = (
        not (np.all(np.asarray(ln1_g) == 1.0) and np.all(np.asarray(ln1_b) == 0.0)),
        not (np.all(np.asarray(ln2_g) == 1.0) and np.all(np.asarray(ln2_b) == 0.0)),
        bool(np.any(np.asarray(mlp_b2) != 0.0)),
        not np.any(np.asarray(mlp_b1) != 0.0),
    )

    # conv weights: per global tap t -> W[:, :, kt].T  (shape [c, o])
    wct = np.empty((NT, D, D), f32)
    t = 0
    for W in (W3, W5, W7):
        W = np.asarray(W, f32)
        for k in range(W.shape[2]):
            wct[t] = W[:, :, k].T
            t += 1
    # partition-major flat: wc[p, ((t*CD + c)*D + o)] = W_t[c*128+p, o]
    wc = np.ascontiguousarray(
        wct.reshape(NT, CD, 128, D).transpose(2, 0, 1, 3).reshape(128, NT * CD * D)
    ).astype(bf)

    f8 = ml_dtypes.float8_e4m3
    shared = {
        "wc": wc,
        "cb1": np.asarray(b3, f32).reshape(CD, 128),
        "cb2": np.asarray(b5, f32).reshape(CD, 128),
        # MLP weights in fp8 e4m3 (DoubleRow matmuls), host-scaled by 64
        # into e4m3's normal range; the kernel folds 1/64 back in.
        "w1": np.ascontiguousarray(
            np.asarray(mlp_w1, f32).reshape(CD, 128, H).transpose(1, 0, 2)
            * 64.0).astype(f8),
        "b1c": np.asarray(mlp_b1, f32).reshape(JD, 128),
        "w2": np.ascontiguousarray(
            np.asarray(mlp_w2, f32).reshape(JD, 128, D).transpose(1, 0, 2)
            .reshape(128, JD // 2, 2, D) * 64.0).astype(f8),
    }
    if flags[0]:
        shared["g1r"] = np.asarray(ln1_g, f32).reshape(1, D)
        shared["b1r"] = np.asarray(ln1_b, f32).reshape(1, D)
    if flags[1]:
        shared["g2r"] = np.asarray(ln2_g, f32).reshape(1, D)
        shared["b2lr"] = np.asarray(ln2_b, f32).reshape(1, D)
    if flags[2]:
        shared["b2r"] = np.asarray(mlp_b2, f32).reshape(1, D)

    b7f = np.asarray(b7, f32)
    in_maps = []
    for b in range(B):
        xc = x[b].T  # (D, L)
        xcp = np.zeros((CD, 128, PL), f32)
        xcp[:, :, PAD:PAD + L] = xc.reshape(CD, 128, L)
        xbv = (x[b] + b7f[None, :]).reshape(LCH, 128, D)
        # masks for shifts d in (-3,-2,-1,1,2,3), evaluated at output position
        ce = np.zeros(L + 8, np.int32)
        ce[4:4 + L] = chain[b]
        # masks are zero-padded by PAD cols each side (kernel reads aligned
        # [l0-PAD, l0+width+PAD) windows); row order is shift (-3..-1,1..3)
        m = np.zeros((6, PAD + L + PAD), bf)
        for mi, d in enumerate((-3, -2, -1, 1, 2, 3)):
            m[mi, PAD:PAD + L] = (ce[4 + d:4 + d + L] == chain[b]).astype(bf)
        im = {"xcb": xcp.astype(bf), "xb": np.ascontiguousarray(xbv),
              "masks": m, **shared}
        in_maps.append(im)
    return in_maps, flags


def kernel(**inputs):
    from concourse.bass_utils import run_bass_kernel_spmd

    in_maps, flags = _prep_inputs(**inputs)
    if flags not in _CACHE:
        _CACHE[flags] = _build_nc(*flags)
    nc = _CACHE[flags]
    res = run_bass_kernel_spmd(nc, in_maps, list(range(NCORES)))
    return np.stack([res.results[b]["out"] for b in range(B)]).astype(np.float32)

